# revision 1
# baseline (speedup 1.0000x reference)
"""DSS-network GNN kernel for trn2: parameterized builder + host prep.

Sharding: graph-parallel over NC cores (contiguous node ranges). Transposed
activation layout [feature, node]. Per layer:
  1. z2 = x @ Wn computed per-core, AllGathered -> gather table z2_full.
  2. x_sum (subgraph-mean) computed per-core, AllGathered (small).
  3. h2 path (orig-graph conv + BN) computed replicated on every core via a
     dense A_s matmul.
  4. Edge aggregation: per dst-chunk of 128 nodes, indirect-DMA gathers of
     z2 rows + one-hot scatter matmuls accumulating in PSUM together with
     the x @ Wr root term.
  5. BN1 stats AllReduce (tiny), affine + h2-recombine + relu -> x_new.
Readout: subgraph/graph mean pooling + 2-layer MLP, per-core graphs only.
"""
import numpy as np

from concourse import bass, bacc, mybir, tile
from concourse.masks import make_identity

f32 = mybir.dt.float32
i32 = mybir.dt.int32
P = 128
EPS = 1e-5


class Cfg:
    def __init__(self, NC, G, N=128, EMB=128, L=4, TASKS=10, B=9):
        assert EMB == 128 and N == 128
        self.NC, self.G, self.N, self.EMB, self.L, self.TASKS = NC, G, N, EMB, L, TASKS
        self.S = G * N                   # total subgraphs == orig nodes
        self.T = G * N * N               # total batched nodes
        self.TP = self.T // NC           # nodes per core
        self.G_loc = G // NC             # graphs per core
        self.CH = self.TP // P           # dst-chunks per core
        self.SLABS = self.TP // 1024     # 8-chunk slabs per core
        self.B = B                       # gather blocks per chunk (host-set)
        self.NB = self.CH * B            # total gather blocks per core
        self.SGL = self.G_loc * N        # own subgraph-slots


def rep3(ap2d, b, inner, bcast_inner):
    """[P, b]-slice -> 3D AP: bcast_inner: [P, b, inner] with inner step 0
    (each column value repeated `inner` times); else iota-style [P, b, inner]
    with b step 0 (the 2d free dim repeated b times; ap2d must be [P, inner])."""
    pp = ap2d.ap[0]
    if bcast_inner:
        return bass.AP(ap2d.tensor, ap2d.offset, [pp, ap2d.ap[1][:], [0, inner]])
    else:
        return bass.AP(ap2d.tensor, ap2d.offset, [pp, [0, b], ap2d.ap[1][:]])


def build(cfg: Cfg):
    nc = bacc.Bacc("TRN2", target_bir_lowering=False, debug=False,
                   num_devices=cfg.NC)
    L, TP, CH, B, NB, S, SGL, G_loc = (cfg.L, cfg.TP, cfg.CH, cfg.B, cfg.NB,
                                       cfg.S, cfg.SGL, cfg.G_loc)
    T = cfg.T
    TASKS = cfg.TASKS
    SLAB = 1024
    NSLAB = TP // SLAB
    CPS = SLAB // P                       # chunks per slab = 8
    rg = [list(range(cfg.NC))]
    shared = "Shared" if cfg.NC > 4 else "Local"

    def din(name, shape, dt=f32):
        return nc.dram_tensor(name, shape, dt, kind="ExternalInput").ap()

    xT0 = din("xT0", [P, TP])
    gidx = din("gidx", [P, NB], i32)
    gdst = din("gdst", [P, NB])
    As = din("As", [S, S])
    h2sel = din("h2sel", [S, SGL])
    Wr = din("Wr", [L * P, P]); Wn = din("Wn", [L * P, P])
    bia = din("bia", [L * P, 1]); gam = din("gam", [L * P, 1]); bet = din("bet", [L * P, 1])
    Wrs = din("Wrs", [L * P, P]); Wns = din("Wns", [L * P, P])
    bias_ = din("bias_", [L * P, 1]); gams = din("gams", [L * P, 1]); bets = din("bets", [L * P, 1])
    Wf1 = din("Wf1", [P, 2 * P]); bf1c = din("bf1c", [2 * P, 1])
    Wf2 = din("Wf2", [2 * P, TASKS]); bf2c = din("bf2c", [TASKS, 1])
    out = nc.dram_tensor("out", [G_loc, TASKS], f32, kind="ExternalOutput").ap()

    with tile.TileContext(nc) as tc:
        with (
            tc.tile_pool(name="const", bufs=1) as cst,
            tc.tile_pool(name="wts", bufs=1) as wts,
            tc.tile_pool(name="big", bufs=1) as big,
            tc.tile_pool(name="io", bufs=2) as io,
            tc.tile_pool(name="xg", bufs=6) as xgp,
            tc.tile_pool(name="oh", bufs=3) as ohp,
            tc.tile_pool(name="sm", bufs=1) as sm,
            tc.tile_pool(name="ps_m", bufs=3, space="PSUM") as ps_m,
            tc.tile_pool(name="ps_t", bufs=2, space="PSUM") as ps_t,
            tc.tile_pool(name="ps_h", bufs=2, space="PSUM") as ps_h,
            tc.tile_pool(name="dram", bufs=1, space="DRAM") as dram,
        ):
            # ------- persistent DRAM -------
            xT_a = dram.tile([P, TP], f32)
            xT_b = dram.tile([P, TP], f32)
            z2_stage = dram.tile([TP, P], f32)
            xsum_bounce = dram.tile([SGL, P], f32)
            stats_bounce = dram.tile([P, 2], f32)
            h1_pre = dram.tile([P, TP], f32)
            z2_fulls, xsum_fulls, stats_reds = [], [], []
            for _li in range(L):
                z2_fulls.append(dram.tile([T, P], f32, addr_space=shared,
                                          name=f"z2_full_{_li}"))
                xsum_fulls.append(dram.tile([S, P], f32, addr_space=shared,
                                            name=f"xsum_full_{_li}"))
                stats_reds.append(dram.tile([P, 2], f32, addr_space=shared,
                                            name=f"stats_red_{_li}"))

            # ------- static SBUF -------
            ident = cst.tile([P, P], f32)
            make_identity(nc, ident[:])
            iota_i = cst.tile([P, P], i32)
            nc.gpsimd.iota(iota_i[:], pattern=[[1, P]], base=0, channel_multiplier=0)
            iota_f = cst.tile([P, P], f32)
            nc.vector.tensor_copy(out=iota_f[:], in_=iota_i[:])
            gidx_sb = cst.tile([P, NB], i32)
            nc.sync.dma_start(out=gidx_sb[:], in_=gidx[:, :])
            gdst_sb = cst.tile([P, NB], f32)
            nc.sync.dma_start(out=gdst_sb[:], in_=gdst[:, :])
            h2sel_sb = cst.tile([P, S // P, SGL], f32)
            nc.sync.dma_start(out=h2sel_sb[:], in_=h2sel[:, :].rearrange(
                "(t p) j -> p t j", p=P))

            eps_col = cst.tile([P, 1], f32)
            nc.vector.memset(eps_col[:], EPS)
            h2own = cst.tile([P, SGL], f32)       # h2T own-graph columns
            hsub = cst.tile([P, SGL], f32)        # readout accumulator

            for li in range(L):
                z2_full, xsum_full, stats_red = (z2_fulls[li], xsum_fulls[li],
                                                 stats_reds[li])
                x_cur = xT0 if li == 0 else (xT_a if li % 2 == 1 else xT_b)[:]
                x_nxt = (xT_a if li % 2 == 0 else xT_b)[:]
                wsl = slice(li * P, (li + 1) * P)

                Wn_t = wts.tile([P, P], f32, tag="Wn_t")
                nc.sync.dma_start(out=Wn_t[:], in_=Wn[wsl, :])
                Wr_t = wts.tile([P, P], f32, tag="Wr_t")
                nc.sync.dma_start(out=Wr_t[:], in_=Wr[wsl, :])
                Wns_t = wts.tile([P, P], f32, tag="Wns_t")
                nc.sync.dma_start(out=Wns_t[:], in_=Wns[wsl, :])
                Wrs_t = wts.tile([P, P], f32, tag="Wrs_t")
                nc.sync.dma_start(out=Wrs_t[:], in_=Wrs[wsl, :])
                vecs = {}
                for nm, src in (("b", bia), ("g", gam), ("be", bet),
                                ("bs", bias_), ("gs", gams), ("bes", bets)):
                    v = wts.tile([P, 1], f32, tag=f"v_{nm}")
                    nc.sync.dma_start(out=v[:], in_=src[wsl, :])
                    vecs[nm] = v

                # ---------- pass 1: z2 + x_sum ----------
                xsum_acc = sm.tile([P, SGL], f32, tag="xsum_acc")
                nc.vector.memset(xsum_acc[:], 0.0)
                for sl in range(NSLAB):
                    xt = io.tile([P, SLAB], f32, tag="xt1")
                    nc.sync.dma_start(out=xt[:], in_=x_cur[:, sl * SLAB:(sl + 1) * SLAB])
                    z2st = io.tile([P, CPS, P], f32, tag="z2st")
                    for j in range(CPS):
                        pz = ps_t.tile([P, P], f32, tag="pst")
                        nc.tensor.matmul(out=pz[:], lhsT=xt[:, j * P:(j + 1) * P],
                                         rhs=Wn_t[:], start=True, stop=True)
                        nc.scalar.activation(out=z2st[:, j, :], in_=pz[:],
                                             func=mybir.ActivationFunctionType.Copy)
                    nc.sync.dma_start(
                        out=z2_stage[:].rearrange("(a j p) f -> p (a j) f", p=P, j=CPS)[
                            :, sl * CPS:(sl + 1) * CPS, :],
                        in_=z2st[:])
                    # x_sum accumulate (reduce over subgraph axis of the slab)
                    red = sm.tile([P, P], f32, tag="red")
                    nc.vector.tensor_reduce(
                        out=red[:], in_=xt[:].rearrange("p (s n) -> p n s", s=CPS),
                        axis=mybir.AxisListType.X, op=mybir.AluOpType.add)
                    gl = sl // (NSLAB // G_loc)
                    nc.vector.tensor_add(out=xsum_acc[:, gl * P:(gl + 1) * P],
                                         in0=xsum_acc[:, gl * P:(gl + 1) * P],
                                         in1=red[:])

                # x_sum -> non-T bounce (scale 1/N), AllGathers
                xso = sm.tile([P, G_loc, P], f32, tag="xso")
                for gl in range(G_loc):
                    pt = ps_t.tile([P, P], f32, tag="pst")
                    nc.tensor.transpose(out=pt[:], in_=xsum_acc[:, gl * P:(gl + 1) * P],
                                        identity=ident[:])
                    nc.scalar.activation(out=xso[:, gl, :], in_=pt[:],
                                         func=mybir.ActivationFunctionType.Copy,
                                         scale=1.0 / cfg.N)
                nc.sync.dma_start(
                    out=xsum_bounce[:].rearrange("(g p) f -> p g f", p=P), in_=xso[:])
                nc.gpsimd.collective_compute(
                    "AllGather", mybir.AluOpType.bypass, replica_groups=rg,
                    ins=[z2_stage[:]], outs=[z2_full[:]])
                nc.gpsimd.collective_compute(
                    "AllGather", mybir.AluOpType.bypass, replica_groups=rg,
                    ins=[xsum_bounce[:]], outs=[xsum_full[:]])

                # ---------- h2 path (replicated) ----------
                NT = S // P
                xsf = sm.tile([P, NT, P], f32, tag="xsf")
                nc.sync.dma_start(out=xsf[:],
                                  in_=xsum_full[:].rearrange("(t p) f -> p t f", p=P))
                xsT = sm.tile([P, S], f32, tag="xsT")
                for t in range(NT):
                    pt = ps_t.tile([P, P], f32, tag="pst")
                    nc.tensor.transpose(out=pt[:], in_=xsf[:, t, :], identity=ident[:])
                    nc.scalar.activation(out=xsT[:, t * P:(t + 1) * P], in_=pt[:],
                                         func=mybir.ActivationFunctionType.Copy)
                zs = sm.tile([P, NT, P], f32, tag="zs")
                for t in range(NT):
                    pz = ps_t.tile([P, P], f32, tag="pst")
                    nc.tensor.matmul(out=pz[:], lhsT=xsT[:, t * P:(t + 1) * P],
                                     rhs=Wns_t[:], start=True, stop=True)
                    nc.scalar.activation(out=zs[:, t, :], in_=pz[:],
                                         func=mybir.ActivationFunctionType.Copy)
                h2pre = sm.tile([P, S], f32, tag="h2pre")
                NQ = S // 512 if S >= 512 else 1
                QW = S // NQ
                for q in range(NQ):
                    ph = ps_h.tile([P, QW], f32, tag="psh")
                    nc.tensor.matmul(out=ph[:], lhsT=Wrs_t[:],
                                     rhs=xsT[:, q * QW:(q + 1) * QW],
                                     start=True, stop=False)
                    for t in range(NT):
                        ast = io.tile([P, QW], f32, tag="ast")
                        nc.sync.dma_start(out=ast[:],
                                          in_=As[t * P:(t + 1) * P, q * QW:(q + 1) * QW])
                        nc.tensor.matmul(out=ph[:], lhsT=zs[:, t, :], rhs=ast[:],
                                         start=False, stop=(t == NT - 1))
                    nc.scalar.activation(out=h2pre[:, q * QW:(q + 1) * QW], in_=ph[:],
                                         func=mybir.ActivationFunctionType.Identity,
                                         bias=vecs["bs"][:])
                # BN2 (replicated, exact)
                s1 = sm.tile([P, 1], f32, tag="s1")
                nc.vector.tensor_reduce(out=s1[:], in_=h2pre[:],
                                        axis=mybir.AxisListType.X, op=mybir.AluOpType.add)
                sq = sm.tile([P, S], f32, tag="h2n")
                nc.vector.tensor_tensor(out=sq[:], in0=h2pre[:], in1=h2pre[:],
                                        op=mybir.AluOpType.mult)
                s2 = sm.tile([P, 1], f32, tag="s2")
                nc.vector.tensor_reduce(out=s2[:], in_=sq[:],
                                        axis=mybir.AxisListType.X, op=mybir.AluOpType.add)
                mu2 = sm.tile([P, 1], f32, tag="mu2")
                nc.vector.tensor_scalar_mul(out=mu2[:], in0=s1[:], scalar1=1.0 / S)
                ex2 = sm.tile([P, 1], f32, tag="ex2")
                nc.vector.tensor_scalar_mul(out=ex2[:], in0=s2[:], scalar1=1.0 / S)
                musq = sm.tile([P, 1], f32, tag="musq")
                nc.vector.tensor_tensor(out=musq[:], in0=mu2[:], in1=mu2[:],
                                        op=mybir.AluOpType.mult)
                var2 = sm.tile([P, 1], f32, tag="var2")
                nc.vector.tensor_tensor(out=var2[:], in0=ex2[:], in1=musq[:],
                                        op=mybir.AluOpType.subtract)
                sd2 = sm.tile([P, 1], f32, tag="sd2")
                nc.scalar.activation(out=sd2[:], in_=var2[:],
                                     func=mybir.ActivationFunctionType.Sqrt, bias=eps_col[:])
                rstd2 = sm.tile([P, 1], f32, tag="rstd2")
                nc.vector.reciprocal(out=rstd2[:], in_=sd2[:])
                A2 = sm.tile([P, 1], f32, tag="A2")
                nc.vector.tensor_tensor(out=A2[:], in0=vecs["gs"][:], in1=rstd2[:],
                                        op=mybir.AluOpType.mult)
                muA2 = sm.tile([P, 1], f32, tag="muA2")
                nc.vector.tensor_tensor(out=muA2[:], in0=mu2[:], in1=A2[:],
                                        op=mybir.AluOpType.mult)
                C2 = sm.tile([P, 1], f32, tag="C2")
                nc.vector.tensor_tensor(out=C2[:], in0=vecs["bes"][:], in1=muA2[:],
                                        op=mybir.AluOpType.subtract)
                h2t = sm.tile([P, S], f32, tag="h2t")
                nc.vector.tensor_scalar(out=h2t[:], in0=h2pre[:], scalar1=A2[:],
                                        scalar2=C2[:], op0=mybir.AluOpType.mult,
                                        op1=mybir.AluOpType.add)
                # own-graph columns of h2t via selection matmul
                h2n = sm.tile([P, NT, P], f32, tag="h2n")
                for t in range(NT):
                    pt = ps_t.tile([P, P], f32, tag="pst")
                    nc.tensor.transpose(out=pt[:], in_=h2t[:, t * P:(t + 1) * P],
                                        identity=ident[:])
                    nc.scalar.activation(out=h2n[:, t, :], in_=pt[:],
                                         func=mybir.ActivationFunctionType.Copy)
                pown = ps_h.tile([P, SGL], f32, tag="psh")
                for t in range(NT):
                    nc.tensor.matmul(out=pown[:], lhsT=h2n[:, t, :],
                                     rhs=h2sel_sb[:, t, :],
                                     start=(t == 0), stop=(t == NT - 1))
                nc.scalar.activation(out=h2own[:], in_=pown[:],
                                     func=mybir.ActivationFunctionType.Copy)

                # ---------- pass 2: gather + scatter ----------
                ssum = sm.tile([P, NSLAB], f32, tag="ssum")
                ssq = sm.tile([P, NSLAB], f32, tag="ssq")
                for sl in range(NSLAB):
                    xt2 = io.tile([P, SLAB], f32, tag="xt2")
                    nc.sync.dma_start(out=xt2[:], in_=x_cur[:, sl * SLAB:(sl + 1) * SLAB])
                    h1st = io.tile([P, CPS, P], f32, tag="h1st")
                    for j in range(CPS):
                        c = sl * CPS + j
                        oh = ohp.tile([P, B, P], f32, tag="oh")
                        nc.vector.tensor_tensor(
                            out=oh[:],
                            in0=rep3(gdst_sb[:, c * B:(c + 1) * B], B, P, True),
                            in1=rep3(iota_f[:], B, P, False),
                            op=mybir.AluOpType.is_equal)
                        pm = ps_m.tile([P, P], f32, tag="pm")
                        for b in range(B):
                            xg = xgp.tile([P, P], f32, tag="xg")
                            nc.gpsimd.indirect_dma_start(
                                out=xg[:], out_offset=None, in_=z2_full[:],
                                in_offset=bass.IndirectOffsetOnAxis(
                                    ap=gidx_sb[:, c * B + b:c * B + b + 1], axis=0))
                            nc.tensor.matmul(out=pm[:], lhsT=xg[:], rhs=oh[:, b, :],
                                             start=(b == 0), stop=False)
                        nc.tensor.matmul(out=pm[:], lhsT=Wr_t[:],
                                         rhs=xt2[:, j * P:(j + 1) * P],
                                         start=False, stop=True)
                        nc.scalar.activation(out=h1st[:, j, :], in_=pm[:],
                                             func=mybir.ActivationFunctionType.Identity,
                                             bias=vecs["b"][:])
                    nc.sync.dma_start(out=h1_pre[:, sl * SLAB:(sl + 1) * SLAB],
                                      in_=h1st[:].rearrange("p a b -> p (a b)"))
                    # stats
                    rs = sm.tile([P, 1], f32, tag="rs")
                    nc.vector.tensor_reduce(
                        out=rs[:], in_=h1st[:].rearrange("p a b -> p (a b)"),
                        axis=mybir.AxisListType.X, op=mybir.AluOpType.add)
                    nc.vector.tensor_copy(out=ssum[:, sl:sl + 1], in_=rs[:])
                    sqt = io.tile([P, SLAB], f32, tag="xf")
                    nc.vector.tensor_tensor(
                        out=sqt[:], in0=h1st[:].rearrange("p a b -> p (a b)"),
                        in1=h1st[:].rearrange("p a b -> p (a b)"),
                        op=mybir.AluOpType.mult)
                    rq = sm.tile([P, 1], f32, tag="rq")
                    nc.vector.tensor_reduce(out=rq[:], in_=sqt[:],
                                            axis=mybir.AxisListType.X,
                                            op=mybir.AluOpType.add)
                    nc.vector.tensor_copy(out=ssq[:, sl:sl + 1], in_=rq[:])

                # ---------- BN1 stats AllReduce ----------
                stv = sm.tile([P, 2], f32, tag="stv")
                nc.vector.tensor_reduce(out=stv[:, 0:1], in_=ssum[:],
                                        axis=mybir.AxisListType.X, op=mybir.AluOpType.add)
                nc.vector.tensor_reduce(out=stv[:, 1:2], in_=ssq[:],
                                        axis=mybir.AxisListType.X, op=mybir.AluOpType.add)
                nc.sync.dma_start(out=stats_bounce[:], in_=stv[:])
                nc.gpsimd.collective_compute(
                    "AllReduce", mybir.AluOpType.add, replica_groups=rg,
                    ins=[stats_bounce[:]], outs=[stats_red[:]])
                str_ = sm.tile([P, 2], f32, tag="str_")
                nc.sync.dma_start(out=str_[:], in_=stats_red[:])
                mu1 = sm.tile([P, 1], f32, tag="mu1")
                nc.vector.tensor_scalar_mul(out=mu1[:], in0=str_[:, 0:1], scalar1=1.0 / T)
                ex1 = sm.tile([P, 1], f32, tag="ex1")
                nc.vector.tensor_scalar_mul(out=ex1[:], in0=str_[:, 1:2], scalar1=1.0 / T)
                mus1 = sm.tile([P, 1], f32, tag="mus1")
                nc.vector.tensor_tensor(out=mus1[:], in0=mu1[:], in1=mu1[:],
                                        op=mybir.AluOpType.mult)
                var1 = sm.tile([P, 1], f32, tag="var1")
                nc.vector.tensor_tensor(out=var1[:], in0=ex1[:], in1=mus1[:],
                                        op=mybir.AluOpType.subtract)
                sd1 = sm.tile([P, 1], f32, tag="sd1")
                nc.scalar.activation(out=sd1[:], in_=var1[:],
                                     func=mybir.ActivationFunctionType.Sqrt, bias=eps_col[:])
                rstd1 = sm.tile([P, 1], f32, tag="rstd1")
                nc.vector.reciprocal(out=rstd1[:], in_=sd1[:])
                A1 = sm.tile([P, 1], f32, tag="A1")
                nc.vector.tensor_tensor(out=A1[:], in0=vecs["g"][:], in1=rstd1[:],
                                        op=mybir.AluOpType.mult)
                muA1 = sm.tile([P, 1], f32, tag="muA1")
                nc.vector.tensor_tensor(out=muA1[:], in0=mu1[:], in1=A1[:],
                                        op=mybir.AluOpType.mult)
                sh1 = sm.tile([P, 1], f32, tag="sh1")
                nc.vector.tensor_tensor(out=sh1[:], in0=vecs["be"][:], in1=muA1[:],
                                        op=mybir.AluOpType.subtract)
                Cg = sm.tile([P, SGL], f32, tag="Cg")
                nc.vector.tensor_scalar_add(out=Cg[:], in0=h2own[:], scalar1=sh1[:])

                # ---------- pass 3: affine + recombine + relu ----------
                for sl in range(NSLAB):
                    h1s = io.tile([P, SLAB], f32, tag="h1s")
                    nc.sync.dma_start(out=h1s[:], in_=h1_pre[:, sl * SLAB:(sl + 1) * SLAB])
                    nc.vector.tensor_scalar_mul(out=h1s[:], in0=h1s[:], scalar1=A1[:])
                    gl = sl // (NSLAB // G_loc)
                    nc.vector.tensor_tensor(
                        out=h1s[:].rearrange("p (a b) -> p a b", a=CPS),
                        in0=h1s[:].rearrange("p (a b) -> p a b", a=CPS),
                        in1=rep3(Cg[:, gl * P:(gl + 1) * P], CPS, P, False),
                        op=mybir.AluOpType.add)
                    nc.vector.tensor_scalar_max(out=h1s[:], in0=h1s[:], scalar1=0.0)
                    nc.sync.dma_start(out=x_nxt[:, sl * SLAB:(sl + 1) * SLAB], in_=h1s[:])

            # ---------- readout ----------
            x_fin = (xT_a if L % 2 == 1 else xT_b)[:]
            for sl in range(NSLAB):
                xf = io.tile([P, SLAB], f32, tag="xf")
                nc.sync.dma_start(out=xf[:], in_=x_fin[:, sl * SLAB:(sl + 1) * SLAB])
                nc.vector.tensor_reduce(
                    out=hsub[:, sl * CPS:(sl + 1) * CPS],
                    in_=xf[:].rearrange("p (c n) -> p c n", c=CPS),
                    axis=mybir.AxisListType.X, op=mybir.AluOpType.add)
            hg = sm.tile([P, G_loc], f32, tag="hg")
            nc.vector.tensor_reduce(
                out=hg[:], in_=hsub[:].rearrange("p (g s) -> p g s", g=G_loc),
                axis=mybir.AxisListType.X, op=mybir.AluOpType.add)
            nc.vector.tensor_scalar_mul(out=hg[:], in0=hg[:],
                                        scalar1=1.0 / (cfg.N * cfg.N))
            Wf1_sb = sm.tile([P, 2 * P], f32, tag="Wf1_sb")
            nc.sync.dma_start(out=Wf1_sb[:], in_=Wf1[:, :])
            Wf2_sb = sm.tile([P, 2, TASKS], f32, tag="Wf2_sb")
            nc.sync.dma_start(out=Wf2_sb[:],
                              in_=Wf2[:, :].rearrange("(a p) t -> p a t", p=P))
            bf1_sb = sm.tile([P, 2], f32, tag="bf1_sb")
            nc.sync.dma_start(out=bf1_sb[:],
                              in_=bf1c[:, :].rearrange("(a p) o -> p (a o)", p=P))
            bf2_sb = sm.tile([TASKS, 1], f32, tag="bf2_sb")
            nc.sync.dma_start(out=bf2_sb[:], in_=bf2c[:, :])
            o1 = sm.tile([P, 2, G_loc], f32, tag="o1")
            for h in range(2):
                p1 = ps_t.tile([P, G_loc], f32, tag="pst")
                nc.tensor.matmul(out=p1[:], lhsT=Wf1_sb[:, h * P:(h + 1) * P],
                                 rhs=hg[:], start=True, stop=True)
                nc.scalar.activation(out=o1[:, h, :], in_=p1[:],
                                     func=mybir.ActivationFunctionType.Relu,
                                     bias=bf1_sb[:, h:h + 1])
            p2 = ps_t.tile([TASKS, G_loc], f32, tag="pst")
            for h in range(2):
                nc.tensor.matmul(out=p2[:], lhsT=Wf2_sb[:, h, :], rhs=o1[:, h, :],
                                 start=(h == 0), stop=(h == 1))
            oT = sm.tile([TASKS, G_loc], f32, tag="oT")
            nc.scalar.activation(out=oT[:], in_=p2[:],
                                 func=mybir.ActivationFunctionType.Identity,
                                 bias=bf2_sb[:])
            nc.sync.dma_start(out=out[:, :].rearrange("a b -> b a"), in_=oT[:])

    nc.compile()
    return nc


def host_prep(inputs, cfg: Cfg):
    """Full inputs dict -> (in_maps list per core, postprocess info)."""
    NC, G, N, L, S, T, TP = cfg.NC, cfg.G, cfg.N, cfg.L, cfg.S, cfg.T, cfg.TP
    x = np.asarray(inputs["x"], np.float32)
    ei = np.asarray(inputs["edge_index"])
    oei = np.asarray(inputs["original_edge_index"])
    batch = np.asarray(inputs["batch"])
    sni = np.asarray(inputs["subgraph_node_idx"])
    sb = np.asarray(inputs["subgraph_batch"])
    nnps = np.asarray(inputs["num_nodes_per_subgraph"])
    sib = np.asarray(inputs["subgraph_idx_batch"])

    # verify the structured DSS layout this kernel hardcodes
    assert np.array_equal(batch, np.repeat(np.arange(G), N * N))
    assert np.array_equal(sni, np.tile(np.arange(N), S))
    assert np.array_equal(sb, np.repeat(np.arange(S), N))
    assert np.all(nnps == N)
    assert np.array_equal(sib, np.repeat(np.arange(G), N))

    As = np.zeros((S, S), np.float32)
    np.add.at(As, (oei[0], oei[1]), 1.0)

    src, dst = ei[0].astype(np.int64), ei[1].astype(np.int64)
    core = dst // TP
    # B: max blocks needed by any (core, chunk)
    chunk_gl = dst // P
    cnt = np.bincount(chunk_gl, minlength=T // P)
    B = max(1, int(np.ceil(cnt.max() / P)))
    cfg.B = B
    cfg.NB = cfg.CH * B

    def stack(w):  # [L,128,128] -> [L*128,128]
        return np.asarray(w, np.float32).reshape(L * P, P)

    def col(v):   # [L,128] -> [L*128,1]
        return np.asarray(v, np.float32).reshape(L * P, 1)

    common = dict(
        As=As,
        Wr=stack(inputs["Wr"]), Wn=stack(inputs["Wn"]),
        bia=col(inputs["b"]), gam=col(inputs["gamma"]), bet=col(inputs["beta"]),
        Wrs=stack(inputs["Wr_s"]), Wns=stack(inputs["Wn_s"]),
        bias_=col(inputs["b_s"]), gams=col(inputs["gamma_s"]), bets=col(inputs["beta_s"]),
        Wf1=np.asarray(inputs["Wf1"], np.float32),
        bf1c=np.asarray(inputs["bf1"], np.float32).reshape(2 * P, 1),
        Wf2=np.asarray(inputs["Wf2"], np.float32),
        bf2c=np.asarray(inputs["bf2"], np.float32).reshape(cfg.TASKS, 1),
    )

    in_maps = []
    for k in range(NC):
        sel = core == k
        s_k = src[sel]
        d_k = dst[sel] - k * TP
        order = np.argsort(d_k, kind="stable")
        s_k, d_k = s_k[order], d_k[order]
        ch = d_k // P
        cnt_k = np.bincount(ch, minlength=cfg.CH)
        starts = np.concatenate([[0], np.cumsum(cnt_k)])[:-1]
        pos_in_chunk = np.arange(len(d_k)) - starts[ch]
        slot = ch * (B * P) + pos_in_chunk
        gi = np.zeros(cfg.CH * B * P, np.int32)
        gd = np.full(cfg.CH * B * P, -1.0, np.float32)
        gi[slot] = s_k
        gd[slot] = (d_k % P).astype(np.float32)
        gi = gi.reshape(cfg.NB, P).T.copy()
        gd = gd.reshape(cfg.NB, P).T.copy()

        h2sel = np.zeros((S, cfg.SGL), np.float32)
        own = np.arange(k * cfg.SGL, (k + 1) * cfg.SGL)
        h2sel[own, np.arange(cfg.SGL)] = 1.0

        m = dict(common)
        m.update(
            xT0=np.ascontiguousarray(x[k * TP:(k + 1) * TP].T),
            gidx=gi, gdst=gd, h2sel=h2sel,
        )
        in_maps.append(m)
    return in_maps




_CACHE = {}


def kernel(**inputs):
    """Full (unsharded) inputs -> full [G, TASKS] output, computed on 8
    trn2 NeuronCores via bass."""
    from concourse import bass_utils

    x = np.asarray(inputs["x"], np.float32)
    G = int(np.asarray(inputs["num_nodes_per_subgraph"]).shape[0])
    N = int(np.asarray(inputs["num_nodes_per_subgraph"])[0])
    TASKS = int(np.asarray(inputs["bf2"]).shape[0])
    L = int(np.asarray(inputs["Wr"]).shape[0])
    NC = 8

    cfg = Cfg(NC=NC, G=G, N=N, L=L, TASKS=TASKS)
    in_maps = host_prep(inputs, cfg)

    key = (NC, G, N, L, TASKS, cfg.B)
    if key not in _CACHE:
        _CACHE[key] = build(cfg)
    nc = _CACHE[key]

    res = bass_utils.run_bass_kernel_spmd(
        nc, in_maps, core_ids=list(range(NC)), trace=False)
    out = np.concatenate([res.results[k]["out"] for k in range(NC)], axis=0)
    return out.astype(np.float32)

